# revision 6
# baseline (speedup 1.0000x reference)
"""Trainium2 Bass kernel for the ECG Mamba classifier.

Sharding: 8 cores = (batch 4) x (d_inner halves 2). Within a pair, the two
cores exchange activations twice per layer (AllGather of the xc halves
before the xproj matmul, and of the gated-y halves before the out
projection). All activations live in [channel, time] layout so every
matmul uses the weight as lhsT directly and the SSM scan runs along the
free (time) dimension via tensor_tensor_scan.
"""

import numpy as np

import concourse.bacc as bacc
import concourse.mybir as mybir
import concourse.tile as tile
from concourse.bass_utils import run_bass_kernel_spmd
from concourse.tile_rust import add_dep_helper

# Bias ACT-table selection so Exp and Ln resolve to the one table that
# contains both (natural_log_exp_and_others) instead of ping-ponging between
# exp-only and ln-only tables on every softplus. Entry order (and therefore
# act_func_set_id) is preserved; we only shrink the advertised func sets.
import concourse.bacc as _bacc_mod
import concourse.mybir as _mybir_mod
_orig_gat = _bacc_mod.get_activation_tables

def _gat_biased(arch):
    tables = _orig_gat(arch)
    exp = _mybir_mod.ActivationFunctionType.Exp
    ln = _mybir_mod.ActivationFunctionType.Ln
    if any(exp in s and ln in s for s in tables.values()):
        for name, s in tables.items():
            if not (exp in s and ln in s):
                s.discard(exp)
                s.discard(ln)
    return tables

_bacc_mod.get_activation_tables = _gat_biased

F32 = mybir.dt.float32
F32R = mybir.dt.float32r
BF16 = mybir.dt.bfloat16
AF = mybir.ActivationFunctionType
OP = mybir.AluOpType

B, L, INPUT_SIZE = 4, 2048, 1
D_MODEL, D_STATE, D_CONV = 128, 16, 4
D_INNER = 256
DH = 128  # d_inner half per core
N_LAYERS, NUM_CLASSES = 6, 5
LN_EPS = 1e-5
NC_COUNT = 8
GROUPS = [[0, 1], [2, 3], [4, 5], [6, 7]]
TH = L // 2  # time half for the scan stage

_CACHE = {}


def _pos_encoding(length, d):
    pos = np.arange(length, dtype=np.float32)[:, None]
    div = np.exp(np.arange(0, d, 2, dtype=np.float32) * (-np.log(10000.0) / d))
    pe = np.zeros((length, d), dtype=np.float32)
    pe[:, 0::2] = np.sin(pos * div)
    pe[:, 1::2] = np.cos(pos * div)
    return pe


NO_CC = False     # debug: replace collectives with local DMA copies (wrong math)
SKIP_SCAN = False  # debug: skip the whole n-loop (wrong math)
SKIP_BCAST = False  # debug: scan without B/C broadcasts and y matmuls (wrong math)


def _build(n_layers=N_LAYERS, dump_h=False, repeat=1):
    nc = bacc.Bacc(None, num_devices=NC_COUNT)

    def din(name, shape, dt=F32):
        return nc.dram_tensor(name, shape, dt, kind="ExternalInput")

    x_row = din("x_row", [1, L])
    pe_t = din("pe_t", [D_MODEL, L])
    inp_w = din("inp_w", [1, D_MODEL])
    inp_b = din("inp_b", [D_MODEL, 1])
    ones_col = din("ones_col", [128, 1])
    ones_row = din("ones_row", [1, 128], F32R)
    ones_row32 = din("ones_row32", [1, 128])
    ident = din("ident", [128, 128], BF16)
    sel32 = din("sel32", [2 * D_STATE, 2 * D_STATE * 128], BF16)
    eps_col = din("eps_col", [128, 1])

    wxc = din("wxc", [n_layers, D_MODEL, 8 * DH], BF16)   # conv-scaled in_proj (xc half), 4 shifts
    wz = din("wz", [n_layers, D_MODEL, DH], BF16)         # in_proj z half
    conv_b = din("conv_b", [n_layers, DH, 2])
    xproj_w = din("xproj_w", [n_layers, D_INNER, D_INNER + 2 * D_STATE], BF16)
    dt_w = din("dt_w", [n_layers, D_INNER, DH], BF16)
    dt_b = din("dt_b", [n_layers, DH, 1])
    a_neg = din("a_neg", [n_layers, DH, D_STATE])
    dp = din("dp", [n_layers, DH, 1])
    out_w = din("out_w", [n_layers, D_INNER, D_MODEL], BF16)
    ln_sel = din("ln_sel", [n_layers, 2, 2 * D_MODEL], F32R)
    ln_b = din("ln_b", [n_layers, D_MODEL, 1])

    cls_ln_w = din("cls_ln_w", [D_MODEL, 1])
    cls_ln_b = din("cls_ln_b", [D_MODEL, 1])
    cls_w1 = din("cls_w1", [D_MODEL, D_MODEL // 2])
    cls_b1 = din("cls_b1", [D_MODEL // 2, 1])
    cls_w2 = din("cls_w2", [D_MODEL // 2, NUM_CLASSES])
    cls_b2 = din("cls_b2", [NUM_CLASSES, 1])

    logits = nc.dram_tensor("logits", [NUM_CLASSES, 1], F32, kind="ExternalOutput")
    h_dump = None
    if dump_h:
        h_dump = nc.dram_tensor("h_dump", [D_MODEL, L], F32, kind="ExternalOutput")

    NCH = L // 512  # 512-wide matmul chunks over full L
    NCH_H = TH // 512  # chunks per time half

    with tile.TileContext(nc) as tc:
        with (
            tc.tile_pool(name="const", bufs=1) as cpool,
            tc.tile_pool(name="hstate", bufs=1) as hpool,
            tc.tile_pool(name="wts", bufs=2) as wpool,
            tc.tile_pool(name="act", bufs=2) as apool,
            tc.tile_pool(name="scan", bufs=3) as spool,
            tc.tile_pool(name="scan2", bufs=2) as spool2,
            tc.tile_pool(name="mm", bufs=2, space="PSUM") as mmp,
            tc.tile_pool(name="ypsum", bufs=1, space="PSUM") as ypp,
            tc.tile_pool(name="bcpsum", bufs=2, space="PSUM") as bcp,
            tc.tile_pool(name="dram", bufs=4, space="DRAM") as dpool,
        ):
            # ---- constants ----
            c_ones_col = cpool.tile([128, 1], F32, tag="ones_col")
            nc.sync.dma_start(c_ones_col[:], ones_col[:])
            c_ones_row = cpool.tile([1, 128], F32R, tag="ones_row")
            nc.sync.dma_start(c_ones_row[:], ones_row[:])
            c_ones_row32 = cpool.tile([1, 128], F32, tag="ones_row32")
            nc.sync.dma_start(c_ones_row32[:], ones_row32[:])
            c_ident = cpool.tile([128, 128], BF16, tag="ident")
            nc.sync.dma_start(c_ident[:], ident[:])
            c_sel = cpool.tile([2 * D_STATE, 2 * D_STATE * 128], BF16, tag="sel32")
            nc.sync.dma_start(c_sel[:], sel32[:])
            c_eps = cpool.tile([128, 1], F32, tag="eps_col")
            nc.sync.dma_start(c_eps[:], eps_col[:])
            c_inp_w = cpool.tile([1, D_MODEL], F32, tag="inp_w")
            nc.sync.dma_start(c_inp_w[:], inp_w[:])
            c_inp_b = cpool.tile([D_MODEL, 1], F32, tag="inp_b")
            nc.sync.dma_start(c_inp_b[:], inp_b[:])


            for _rep in range(repeat):
                # ---- h state, padded with 3 zero columns on the left ----
                h_pad = hpool.tile([D_MODEL, L + 3], BF16, tag="h_pad")
                nc.vector.memset(h_pad[:, 0:3], 0.0)
                h = h_pad[:, 3 : 3 + L]

                # ---- embedding: h = x @ inp_w + inp_b + pe ----
                with tc.tile_pool(name="emb", bufs=2) as epool:
                    for c in range(NCH):
                        cs = slice(c * 512, (c + 1) * 512)
                        c_x = epool.tile([1, 512], F32, tag="x_row")
                        nc.sync.dma_start(c_x[:], x_row[:, cs])
                        c_pe = epool.tile([D_MODEL, 512], F32, tag="pe_t")
                        nc.sync.dma_start(c_pe[:], pe_t[:, cs])
                        pm = mmp.tile([128, 512], F32, tag="mm")
                        nc.tensor.matmul(pm[:], c_inp_w[:], c_x[:])
                        nc.scalar.activation(
                            h[:, cs], pm[:], AF.Identity, bias=c_inp_b[:, 0:1],
                        )
                        nc.vector.tensor_add(h[:, cs], h[:, cs], c_pe[:])

                for layer in range(n_layers):
                    # ---- load layer weights ----
                    w_xc = wpool.tile([D_MODEL, 8 * DH], BF16, tag="w_xc")
                    nc.sync.dma_start(w_xc[:], wxc[layer])
                    w_z = wpool.tile([D_MODEL, DH], BF16, tag="w_z")
                    nc.sync.dma_start(w_z[:], wz[layer])
                    w_cb = wpool.tile([DH, 2], F32, tag="w_cb")
                    nc.sync.dma_start(w_cb[:], conv_b[layer])
                    w_xp0 = wpool.tile([128, 288], BF16, tag="w_xp0")
                    nc.sync.dma_start(w_xp0[:], xproj_w[layer, 0:128])
                    w_xp1 = wpool.tile([128, 288], BF16, tag="w_xp1")
                    nc.sync.dma_start(w_xp1[:], xproj_w[layer, 128:256])
                    w_dt0 = wpool.tile([128, DH], BF16, tag="w_dt0")
                    nc.sync.dma_start(w_dt0[:], dt_w[layer, 0:128])
                    w_dt1 = wpool.tile([128, DH], BF16, tag="w_dt1")
                    nc.sync.dma_start(w_dt1[:], dt_w[layer, 128:256])
                    w_dtb = wpool.tile([DH, 1], F32, tag="w_dtb")
                    nc.sync.dma_start(w_dtb[:], dt_b[layer])
                    w_an = wpool.tile([DH, D_STATE], F32, tag="w_an")
                    nc.sync.dma_start(w_an[:], a_neg[layer])
                    w_dp = wpool.tile([DH, 1], F32, tag="w_dp")
                    nc.sync.dma_start(w_dp[:], dp[layer])
                    w_ow0 = wpool.tile([128, D_MODEL], BF16, tag="w_ow0")
                    nc.sync.dma_start(w_ow0[:], out_w[layer, 0:128])
                    w_ow1 = wpool.tile([128, D_MODEL], BF16, tag="w_ow1")
                    nc.sync.dma_start(w_ow1[:], out_w[layer, 128:256])
                    w_lnsel = wpool.tile([2, 2 * D_MODEL], F32R, tag="w_lnsel")
                    nc.sync.dma_start(w_lnsel[:], ln_sel[layer])
                    w_lnb = wpool.tile([D_MODEL, 1], F32, tag="w_lnb")
                    nc.sync.dma_start(w_lnb[:], ln_b[layer])

                    # ---- xz matmuls with fused causal depthwise conv (xc) ----
                    silu_insts = []
                    z_act = apool.tile([DH, L], BF16, tag="z_act")
                    xc_f = [
                        apool.tile([DH, L], BF16, tag="xc_f0", name="xc_f0"),
                        apool.tile([DH, L], BF16, tag="xc_f1", name="xc_f1"),
                    ]
                    for c in range(NCH):
                        for hh in range(2):
                            pm = mmp.tile([128, 512], F32, tag="mm")
                            for j in range(D_CONV):
                                nc.tensor.matmul(
                                    pm[:],
                                    w_xc[:, hh * 4 * DH + j * DH : hh * 4 * DH + (j + 1) * DH],
                                    h_pad[:, c * 512 + j : c * 512 + j + 512],
                                    start=(j == 0),
                                    stop=(j == D_CONV - 1),
                                )
                            _si = nc.scalar.activation(
                                xc_f[hh][:, c * 512 : (c + 1) * 512], pm[:], AF.Silu,
                                bias=(w_cb[:, 0:1] if hh == 0 else w_cb[:, 1:2]),
                            )
                            silu_insts.append(_si)
                        pz = mmp.tile([128, 512], F32, tag="mm")
                        nc.tensor.matmul(pz[:], w_z[:], h[:, c * 512 : (c + 1) * 512])
                        _sz = nc.scalar.activation(
                            z_act[:, c * 512 : (c + 1) * 512], pz[:], AF.Silu
                        )
                        silu_insts.append(_sz)

                    y_f = [
                        apool.tile([DH, L], BF16, tag="y_f0", name="y_f0"),
                        apool.tile([DH, L], BF16, tag="y_f1", name="y_f1"),
                    ]
                    s_state = apool.tile([DH, D_STATE], F32, tag="s_state")

                    for half in range(2):
                        hs = slice(half * TH, (half + 1) * TH)

                        # per-half activation tiles
                        delta_raw = [
                            apool.tile([128, TH], BF16, tag="draw0", name="draw0"),
                            apool.tile([128, TH], BF16, tag="draw1", name="draw1"),
                        ]
                        bc_sb = apool.tile([2 * D_STATE, TH], BF16, tag="bc")
                        delta = apool.tile([DH, TH], BF16, tag="delta")
                        dxu = apool.tile([DH, TH], F32, tag="dxu")

                        # ---- xproj: [256] -> [288] on this time half ----
                        for cc in range(NCH_H):
                            gcs = slice(half * TH + cc * 512, half * TH + (cc + 1) * 512)
                            lcs = slice(cc * 512, (cc + 1) * 512)
                            for mi, msz in ((0, 128), (1, 128), (2, 32)):
                                pm = mmp.tile([128, 512], F32, tag="mm")
                                nc.tensor.matmul(
                                    pm[:msz],
                                    w_xp0[:, mi * 128 : mi * 128 + msz],
                                    xc_f[0][:, gcs],
                                    start=True, stop=False,
                                )
                                nc.tensor.matmul(
                                    pm[:msz],
                                    w_xp1[:, mi * 128 : mi * 128 + msz],
                                    xc_f[1][:, gcs],
                                    start=False, stop=True,
                                )
                                if mi < 2:
                                    nc.scalar.copy(delta_raw[mi][:, lcs], pm[:])
                                else:
                                    nc.scalar.copy(bc_sb[:, lcs], pm[:32])
                            pm = mmp.tile([128, 512], F32, tag="mm")
                            nc.tensor.matmul(
                                pm[:], w_dt0[:], delta_raw[0][:, lcs], start=True, stop=False
                            )
                            nc.tensor.matmul(
                                pm[:], w_dt1[:], delta_raw[1][:, lcs], start=False, stop=True
                            )
                            esp = apool.tile([DH, 512], BF16, tag="esp")
                            _ei = nc.scalar.activation(
                                esp[:], pm[:], AF.Exp, bias=w_dtb[:, 0:1]
                            )
                            for _s in silu_insts:
                                add_dep_helper(
                                    _ei.ins, _s.ins, sync=False,
                                    reason="pin silus before exp (ACT table)",
                                )
                            silu_insts = []
                            nc.scalar.activation(
                                delta[:, lcs], esp[:], AF.Ln, bias=1.0
                            )

                        nc.vector.tensor_mul(dxu[:], delta[:], xc_f[0][:, hs])

                        # ---- scan over the 16 states ----
                        y_acc = ypp.tile([DH, TH], F32, tag="y_acc")
                        for n in range(0 if SKIP_SCAN else D_STATE):
                            da = spool.tile([DH, TH], BF16, tag="da")
                            nc.scalar.activation(
                                da[:], delta[:], AF.Exp, scale=w_an[:, n : n + 1]
                            )
                            if SKIP_BCAST:
                                dbu = dxu
                            else:
                                bb = bcp.tile([128, TH], F32, tag="bc_b")
                                for cc in range(NCH_H):
                                    nc.tensor.matmul(
                                        bb[:, cc * 512 : (cc + 1) * 512],
                                        c_sel[:, n * 128 : (n + 1) * 128],
                                        bc_sb[:, cc * 512 : (cc + 1) * 512],
                                    )
                                dbu = spool.tile([DH, TH], F32, tag="dbu")
                                nc.vector.tensor_mul(dbu[:], dxu[:], bb[:])
                            s_n = spool.tile([DH, TH], F32, tag="s_n")
                            nc.vector.tensor_tensor_scan(
                                s_n[:], da[:], dbu[:],
                                initial=(0.0 if half == 0 else s_state[:, n : n + 1]),
                                op0=OP.mult, op1=OP.add,
                            )
                            if half == 0:
                                nc.scalar.copy(s_state[:, n : n + 1], s_n[:, TH - 1 : TH])
                            if not SKIP_BCAST:
                                cb = bcp.tile([128, TH], F32, tag="bc_b")
                                for cc in range(NCH_H):
                                    nc.tensor.matmul(
                                        cb[:, cc * 512 : (cc + 1) * 512],
                                        c_sel[:, (D_STATE + n) * 128 : (D_STATE + n + 1) * 128],
                                        bc_sb[:, cc * 512 : (cc + 1) * 512],
                                    )
                                w_n = spool2.tile([DH, TH], BF16, tag="w_n")
                                nc.vector.tensor_mul(w_n[:], s_n[:], cb[:])
                                for cc in range(NCH_H):
                                    nc.tensor.matmul(
                                        y_acc[:, cc * 512 : (cc + 1) * 512],
                                        c_ident[:],
                                        w_n[:, cc * 512 : (cc + 1) * 512],
                                        start=(n == 0), stop=(n == D_STATE - 1),
                                    )
                        # y_res = xc*Dp + y_acc ; gate with silu(z); send to partner
                        yr = spool2.tile([DH, TH], BF16, tag="yr")
                        if SKIP_SCAN or SKIP_BCAST:
                            nc.vector.tensor_scalar_mul(yr[:], xc_act[:, hs], w_dp[:, 0:1])
                        else:
                            nc.vector.scalar_tensor_tensor(
                                yr[:], xc_f[0][:, hs], w_dp[:, 0:1], y_acc[:],
                                op0=OP.mult, op1=OP.add,
                            )
                        nc.vector.tensor_mul(yr[:], yr[:], z_act[:, hs])

                        ag2_in = dpool.tile([DH, TH], BF16, tag="ag_y_in")
                        ag2_out = dpool.tile([D_INNER, TH], BF16, tag="ag_y_out")
                        nc.sync.dma_start(ag2_in[:], yr[:])
                        if NO_CC:
                            nc.sync.dma_start(ag2_out[0:DH], ag2_in[:])
                            nc.sync.dma_start(ag2_out[DH:], ag2_in[:])
                        else:
                            nc.gpsimd.collective_compute(
                                "AllGather",
                                OP.bypass,
                                replica_groups=GROUPS,
                                ins=[ag2_in.opt()],
                                outs=[ag2_out.opt()],
                            )
                        nc.sync.dma_start(y_f[0][:, hs], ag2_out[0:DH])
                        nc.sync.dma_start(y_f[1][:, hs], ag2_out[DH:])

                    # ---- out projection + residual + layernorm over d_model ----
                    tmp = apool.tile([D_MODEL, L], F32, tag="ln_tmp")
                    stat_row_sum = apool.tile([1, L], F32, tag="stat_row_sum")
                    stat_row_sq = apool.tile([1, L], F32, tag="stat_row_sq")
                    for c in range(NCH):
                        cs = slice(c * 512, (c + 1) * 512)
                        pm = mmp.tile([128, 512], F32, tag="mm")
                        nc.tensor.matmul(pm[:], w_ow0[:], y_f[0][:, cs], start=True, stop=False)
                        nc.tensor.matmul(pm[:], w_ow1[:], y_f[1][:, cs], start=False, stop=True)
                        nc.vector.tensor_add(tmp[:, cs], pm[:], h[:, cs])
                        sq = apool.tile([D_MODEL, 512], F32, tag="ln_sq")
                        nc.scalar.activation(sq[:], tmp[:, cs], AF.Square)
                        ps = mmp.tile([128, 512], F32, tag="mm")
                        nc.tensor.matmul(ps[0:1], c_ones_col[:], tmp[:, cs])
                        nc.scalar.copy(stat_row_sum[:, cs], ps[0:1])
                        ps2 = ypp.tile([128, 512], F32, tag="y_acc")
                        nc.tensor.matmul(ps2[0:1], c_ones_col[:], sq[:])
                        nc.scalar.copy(stat_row_sq[:, cs], ps2[0:1])
                    # stats: m = sum/128 ; var = sumsq/128 - m^2 ; r = 1/sqrt(var+eps)
                    stat_sq128 = apool.tile([128, 2 * (L // 128)], F32, tag="stat128")
                    nc.sync.dma_start(stat_sq128[:, 0 : L // 128], stat_row_sum[:])
                    nc.sync.dma_start(stat_sq128[:, L // 128 :], stat_row_sq[:])
                    ssum = stat_sq128[:, 0 : L // 128]
                    ssq = stat_sq128[:, L // 128 :]
                    m_t = apool.tile([128, L // 128], F32R, tag="m_t")
                    nc.vector.tensor_scalar_mul(m_t[:], ssum, 1.0 / 128.0)
                    msq = apool.tile([128, L // 128], F32, tag="msq")
                    nc.vector.tensor_mul(msq[:], m_t[:], m_t[:])
                    var = apool.tile([128, L // 128], F32, tag="var")
                    nc.vector.scalar_tensor_tensor(
                        var[:], ssq, 1.0 / 128.0, msq[:], op0=OP.mult, op1=OP.subtract
                    )
                    lnv = apool.tile([128, L // 128], F32, tag="lnv")
                    nc.scalar.activation(lnv[:], var[:], AF.Ln, bias=c_eps[:, 0:1])
                    rstd = apool.tile([128, L // 128], F32R, tag="rstd")
                    nc.scalar.activation(rstd[:], lnv[:], AF.Exp, scale=-0.5)
                    mr_rows = apool.tile([2, L], F32R, tag="mr_rows")
                    nc.sync.dma_start(mr_rows[0:1, :], m_t[:])
                    nc.sync.dma_start(mr_rows[1:2, :], rstd[:])
                    for c in range(NCH):
                        cs = slice(c * 512, (c + 1) * 512)
                        pmb = bcp.tile([128, TH], F32, tag="bc_b")
                        nc.tensor.matmul(
                            pmb[:, 0:512], w_lnsel[:, 0:D_MODEL],
                            mr_rows[:, cs],
                        )
                        prb = bcp.tile([128, TH], F32, tag="bc_b")
                        nc.tensor.matmul(
                            prb[:, 0:512], w_lnsel[:, D_MODEL:],
                            mr_rows[:, cs],
                        )
                        nc.vector.tensor_sub(tmp[:, cs], tmp[:, cs], pmb[:, 0:512])
                        nc.vector.tensor_mul(tmp[:, cs], tmp[:, cs], prb[:, 0:512])
                        nc.vector.tensor_scalar(
                            h[:, cs], tmp[:, cs], w_lnb[:, 0:1], None, op0=OP.add
                        )

                if dump_h:
                    pass  # h_dump disabled for bf16 h

                # ---- classifier ----
                h_sum = cpool.tile([D_MODEL, 1], F32, tag="h_sum")
                nc.vector.tensor_reduce(h_sum[:], h[:], axis=mybir.AxisListType.X, op=OP.add)
                hm = cpool.tile([D_MODEL, 1], F32, tag="hm")
                nc.vector.tensor_scalar_mul(hm[:], h_sum[:], 1.0 / float(L))
                hsq = cpool.tile([D_MODEL, 1], F32, tag="hsq")
                nc.scalar.activation(hsq[:], hm[:], AF.Square)
                ps1 = mmp.tile([128, 512], F32, tag="mm")
                nc.tensor.matmul(ps1[0:1, 0:1], c_ones_col[:], hm[:])
                ps2 = mmp.tile([128, 512], F32, tag="mm")
                nc.tensor.matmul(ps2[0:1, 0:1], c_ones_col[:], hsq[:])
                s1 = cpool.tile([1, 1], F32, tag="s1")
                nc.scalar.copy(s1[:], ps1[0:1, 0:1])
                s2 = cpool.tile([1, 1], F32, tag="s2")
                nc.scalar.copy(s2[:], ps2[0:1, 0:1])
                cm = cpool.tile([1, 1], F32, tag="cm")
                nc.vector.tensor_scalar_mul(cm[:], s1[:], 1.0 / 128.0)
                cmsq = cpool.tile([1, 1], F32, tag="cmsq")
                nc.vector.tensor_mul(cmsq[:], cm[:], cm[:])
                cvar = cpool.tile([1, 1], F32, tag="cvar")
                nc.vector.scalar_tensor_tensor(
                    cvar[:], s2[:], 1.0 / 128.0, cmsq[:], op0=OP.mult, op1=OP.subtract
                )
                clnv = cpool.tile([1, 1], F32, tag="clnv")
                nc.scalar.activation(clnv[:], cvar[:], AF.Ln, bias=c_eps[0:1, 0:1])
                crstd = cpool.tile([1, 1], F32, tag="crstd")
                nc.scalar.activation(crstd[:], clnv[:], AF.Exp, scale=-0.5)
                pmb = mmp.tile([128, 512], F32, tag="mm")
                nc.tensor.matmul(pmb[:, 0:1], c_ones_row32[:], cm[:])
                prb = mmp.tile([128, 512], F32, tag="mm")
                nc.tensor.matmul(prb[:, 0:1], c_ones_row32[:], crstd[:])
                c_lnw = cpool.tile([D_MODEL, 1], F32, tag="c_lnw")
                nc.sync.dma_start(c_lnw[:], cls_ln_w[:])
                c_lnb = cpool.tile([D_MODEL, 1], F32, tag="c_lnb")
                nc.sync.dma_start(c_lnb[:], cls_ln_b[:])
                un = cpool.tile([D_MODEL, 1], F32, tag="un")
                nc.vector.tensor_sub(un[:], hm[:], pmb[:, 0:1])
                nc.vector.tensor_mul(un[:], un[:], prb[:, 0:1])
                hn = cpool.tile([D_MODEL, 1], F32, tag="hn")
                nc.vector.tensor_scalar(
                    hn[:], un[:], c_lnw[:, 0:1], c_lnb[:, 0:1], op0=OP.mult, op1=OP.add
                )
                c_w1 = cpool.tile([D_MODEL, D_MODEL // 2], F32, tag="c_w1")
                nc.sync.dma_start(c_w1[:], cls_w1[:])
                c_b1 = cpool.tile([D_MODEL // 2, 1], F32, tag="c_b1")
                nc.sync.dma_start(c_b1[:], cls_b1[:])
                c_w2 = cpool.tile([D_MODEL // 2, NUM_CLASSES], F32, tag="c_w2")
                nc.sync.dma_start(c_w2[:], cls_w2[:])
                c_b2 = cpool.tile([NUM_CLASSES, 1], F32, tag="c_b2")
                nc.sync.dma_start(c_b2[:], cls_b2[:])
                p1 = mmp.tile([128, 512], F32, tag="mm")
                nc.tensor.matmul(p1[0 : D_MODEL // 2, 0:1], c_w1[:], hn[:])
                r1 = cpool.tile([D_MODEL // 2, 1], F32, tag="r1")
                nc.scalar.activation(r1[:], p1[0 : D_MODEL // 2, 0:1], AF.Relu, bias=c_b1[:, 0:1])
                p2 = mmp.tile([128, 512], F32, tag="mm")
                nc.tensor.matmul(p2[0:NUM_CLASSES, 0:1], c_w2[:], r1[:])
                lg = cpool.tile([NUM_CLASSES, 1], F32, tag="lg")
                nc.scalar.activation(lg[:], p2[0:NUM_CLASSES, 0:1], AF.Identity, bias=c_b2[:, 0:1])
                nc.sync.dma_start(logits[:], lg[:])

    nc.finalize()
    return nc


def _prep_inputs(inputs, n_layers=N_LAYERS):
    import ml_dtypes
    bf = lambda a: np.ascontiguousarray(a).astype(ml_dtypes.bfloat16)
    """Host-side weight prep: per-core input maps (core = 2*b + h)."""
    f = lambda a: np.ascontiguousarray(np.asarray(a), dtype=np.float32)
    x = f(inputs["x"])
    in_proj_w = f(inputs["in_proj_w"])
    conv_w = f(inputs["conv_w"])
    pe = _pos_encoding(L, D_MODEL).T.copy()  # [128, L]
    sel32_np = np.zeros((2 * D_STATE, 2 * D_STATE * 128), np.float32)
    for n in range(2 * D_STATE):
        sel32_np[n, n * 128 : (n + 1) * 128] = 1.0
    ln_w = f(inputs["ln_w"])
    ln_sel_np = np.zeros((n_layers, 2, 2 * D_MODEL), np.float32)
    ln_sel_np[:, 0, 0:D_MODEL] = 1.0
    ln_sel_np[:, 1, D_MODEL:] = ln_w[:n_layers]

    maps = []
    for core in range(NC_COUNT):
        b, h = core // 2, core % 2
        sl = slice(h * DH, (h + 1) * DH)
        slo = slice((1 - h) * DH, (2 - h) * DH)  # partner half
        wxc_np = np.empty((n_layers, D_MODEL, 8 * DH), np.float32)
        for l in range(n_layers):
            for hh, s_ in ((0, sl), (1, slo)):
                w_half = in_proj_w[l][:, s_]  # [128, DH]
                for j in range(D_CONV):
                    wxc_np[l, :, hh * 4 * DH + j * DH : hh * 4 * DH + (j + 1) * DH] = (
                        w_half * conv_w[l, s_, 0, j][None, :]
                    )
        m = {
            "x_row": x[b, :, 0][None, :],
            "pe_t": pe,
            "inp_w": f(inputs["inp_w"]).reshape(1, D_MODEL),
            "inp_b": f(inputs["inp_b"]).reshape(D_MODEL, 1),
            "ones_col": np.ones((128, 1), np.float32),
            "ones_row": np.ones((1, 128), np.float32),
            "ones_row32": np.ones((1, 128), np.float32),
            "ident": np.eye(128, dtype=np.float32),  # cast below
            "sel32": sel32_np,  # cast below
            "eps_col": np.full((128, 1), LN_EPS, np.float32),
            "wxc": wxc_np,  # cast below
            "wz": in_proj_w[:n_layers, :, D_INNER + h * DH : D_INNER + (h + 1) * DH],
            "conv_b": np.stack(
                [f(inputs["conv_b"])[:n_layers, sl],
                 f(inputs["conv_b"])[:n_layers, slo]], axis=-1),
            "xproj_w": np.concatenate(
                [f(inputs["xproj_w"])[:n_layers, sl, :],
                 f(inputs["xproj_w"])[:n_layers, slo, :]], axis=1),
            "dt_w": f(inputs["dt_w"])[:n_layers, :, sl],
            "dt_b": f(inputs["dt_b"])[:n_layers, sl, None],
            "a_neg": -np.exp(f(inputs["A_log"])[:n_layers, sl, :]),
            "dp": f(inputs["Dp"])[:n_layers, sl, None],
            "out_w": f(inputs["out_w"])[:n_layers],
            "ln_sel": ln_sel_np,
            "ln_b": f(inputs["ln_b"])[:n_layers, :, None],
            "cls_ln_w": f(inputs["cls_ln_w"]).reshape(D_MODEL, 1),
            "cls_ln_b": f(inputs["cls_ln_b"]).reshape(D_MODEL, 1),
            "cls_w1": f(inputs["cls_w1"]),
            "cls_b1": f(inputs["cls_b1"]).reshape(-1, 1),
            "cls_w2": f(inputs["cls_w2"]),
            "cls_b2": f(inputs["cls_b2"]).reshape(-1, 1),
        }
        bf_keys = {"wxc", "wz", "xproj_w", "dt_w", "out_w", "sel32", "ident"}
        maps.append({k: (bf(v) if k in bf_keys else np.ascontiguousarray(v, np.float32))
                     for k, v in m.items()})
    return maps


def _run(inputs, n_layers=N_LAYERS, dump_h=False, repeat=1):
    key = (n_layers, dump_h, repeat)
    if key not in _CACHE:
        _CACHE[key] = _build(n_layers, dump_h, repeat)
    nc = _CACHE[key]
    maps = _prep_inputs(inputs, n_layers)
    last_err = None
    for _attempt in range(3):
        try:
            return run_bass_kernel_spmd(nc, maps, list(range(NC_COUNT)))
        except Exception as e:  # transient NRT/device errors happen on cold starts
            last_err = e
    raise last_err


def kernel(**inputs):
    res = _run(inputs)
    out = np.zeros((B, NUM_CLASSES), np.float32)
    for b in range(B):
        out[b] = res.results[2 * b]["logits"][:, 0]
    return out

